# revision 49
# baseline (speedup 1.0000x reference)
"""Trainium2 Bass kernel for nn_Attention_48000554500172.

16-head causal attention with RoPE (S=4096, D=2048, H=16, DH=128), sharded
over heads across 8 NeuronCores (2 heads/core, tensor parallel). Each core
computes its heads' QKV projections, RoPE, causal softmax attention and the
partial output projection; the 8 partial [S, D] outputs are summed on host
(the all-reduce of the sharding hint).

v2 design (vs the 452us fp32r baseline):
- All matmul inputs in bf16 (1 cycle/row on the PE at ANY moving size, same
  as fp32r>=256, but: half the DMA bytes, 2-4x DVE element ops, and no
  min-256 moving-width constraint so diagonal blocks shrink to their true
  causal widths). PSUM accumulation stays fp32; measured rel err ~1e-2/2
  budget.
- No fp32->fp32r rounding copies: DMA lands bf16 directly.
- Softmax denominators: instead of ~88 ones-matmuls per head (each costing
  a full 512-row pass on the PE), probs tiles are folded on the DVE (bf16
  quad trees + one f32r running master per head/group, in-place partial
  width adds for the diagonal) and ONE ones-matmul per (head, group)
  computes the column sums. PE cost for Z drops ~30us.
- Phase interleaving: the per-block attention pipeline is exp-paced on the
  ACT engine (612ns/block vs 426ns of PE sim+PV work), so the projection
  matmuls of slice g+1 and the output projection of group g-1 are emitted
  as "fill" between the sweep blocks of group g. The PE never idles waiting
  for exp.
- PSUM: pacc(2: proj jt-pair accumulators) + pshared(3: sim/rope-P/vT/
  bc/outproj rotation) + ppvz(3: pv_h0, z_h0, pv_h1, z_h1 ring) = 8 banks,
  sized so consecutive groups/heads never serialize on banks.
- Startup: slice-0 x chunks and weight chunks DMA'd interleaved in exactly
  d-loop consumption order; first matmul at ~1.5us.
"""
import math
import numpy as np
import ml_dtypes
from collections import deque
from contextlib import ExitStack

import concourse.bass as bass
import concourse.tile as tile
from concourse import bacc, mybir
from concourse.bass_utils import run_bass_kernel_spmd

D, H, DH = 2048, 16, 128
NCORES = 8
HPC = H // NCORES  # 2 heads per core
ROPE_BASE = 10000.0
SCALE = 1.0 / math.sqrt(DH)
F32 = mybir.dt.float32
F32R = mybir.dt.float32r
BF16 = mybir.dt.bfloat16
Exp = mybir.ActivationFunctionType.Exp
BF = ml_dtypes.bfloat16

_BUILD_CACHE: dict = {}
TRACE = False
LAST_RESULT = None

# diagonal key-block p: query cols [128p, 512) are unmasked
DW = (512, 384, 256, 128)
DO = (0, 128, 256, 384)


def _build(S: int):
    assert S % 512 == 0
    NG = S // 512   # 8 query groups == s-slices
    ND = D // 128   # 16 contraction tiles
    NB = S // 128   # 32 key blocks

    nc = bacc.Bacc("TRN2", target_bir_lowering=False, debug=False)

    xT_d = nc.dram_tensor("xT", [D, S], BF16, kind="ExternalInput")
    w_d = nc.dram_tensor("wqkvT", [D, 768], BF16, kind="ExternalInput")
    wo_d = nc.dram_tensor("woT", [2 * DH, D], BF16, kind="ExternalInput")
    cs_d = nc.dram_tensor("cs", [128, 2 * S], BF16, kind="ExternalInput")
    cb_d = nc.dram_tensor("cb", [128, 2304], BF16, kind="ExternalInput")
    cr_d = nc.dram_tensor("cr", [128, 130], F32R, kind="ExternalInput")
    out_d = nc.dram_tensor("outp", [S, D], BF16, kind="ExternalOutput")

    with tile.TileContext(nc) as tc, ExitStack() as ctx:
        persist = ctx.enter_context(tc.tile_pool(name="persist", bufs=1))
        work = ctx.enter_context(tc.tile_pool(name="work", bufs=2))
        pacc = ctx.enter_context(tc.tile_pool(name="pacc", bufs=3, space="PSUM"))
        pshared = ctx.enter_context(tc.tile_pool(name="psh", bufs=3, space="PSUM"))
        ppv = ctx.enter_context(tc.tile_pool(name="ppv", bufs=2, space="PSUM"))

        # ---- persistent tiles ----
        w_r = persist.tile([128, ND * 768], BF16, tag="w", name="wr")
        wo_r = persist.tile([128, 2 * D], BF16, tag="wo", name="wor")
        kT = [persist.tile([128, S], BF16, tag=f"kT{h}", name=f"kT{h}")
              for h in range(2)]
        v_sb = persist.tile([128, NB * 256], BF16, tag="v", name="vsb")
        cb_t = persist.tile([128, 2304], BF16, tag="cb", name="cbt")
        cr_t = persist.tile([128, 130], F32R, tag="cr", name="crt")
        PT_b = cb_t[:, 0:128]
        ident_b = cb_t[:, 128:256]
        onescol_r = cr_t[:, 0:1]
        onesrow_r = cr_t[0:1, 1:129]
        masks = [cb_t[:, 256 + p * 512:256 + p * 512 + DW[p]] for p in range(4)]

        # ---- DMA issue helpers ----
        def issue_x(g, interleave_w=False):
            """DMA x chunks (+cos/sin) for slice g; optionally interleave the
            16 w chunks + cb in d-consumption order (startup)."""
            xs = []
            for dd in range(ND // 2):
                t = work.tile([128, 1024], BF16, tag="xs", bufs=18, name="xs")
                src = xT_d.ap()[dd * 256:(dd + 1) * 256, g * 512:(g + 1) * 512]
                nc.sync.dma_start(
                    t[:].rearrange("b (a c) -> b a c", a=2),
                    src.rearrange("(a b) c -> b a c", a=2),
                )
                xs.append(t)
                if interleave_w:
                    if dd == 0:
                        # singles so the very first matmul unblocks sooner
                        for d in (0, 1):
                            nc.sync.dma_start(
                                w_r[:, d * 768:(d + 1) * 768],
                                w_d.ap()[d * 128:(d + 1) * 128, :],
                            )
                    else:
                        # two w chunks in one strided DMA, consumption order
                        wdst = w_r[:, dd * 1536:(dd + 1) * 1536]
                        wsrc = w_d.ap()[dd * 256:(dd + 1) * 256, :]
                        nc.sync.dma_start(
                            wdst.rearrange("b (a c) -> b a c", a=2),
                            wsrc.rearrange("(a b) c -> b a c", a=2),
                        )
                    if dd == 1:
                        nc.sync.dma_start(cb_t[:], cb_d.ap())
                    if dd == 2:
                        nc.sync.dma_start(cr_t[:], cr_d.ap())
            cos = work.tile([128, 512], BF16, tag="cos", bufs=2, name="cos")
            sin = work.tile([128, 512], BF16, tag="sin", bufs=2, name="sin")
            nc.sync.dma_start(cos[:], cs_d.ap()[:, g * 512:(g + 1) * 512])
            nc.sync.dma_start(sin[:], cs_d.ap()[:, S + g * 512:S + (g + 1) * 512])
            return xs, cos, sin

        # ---- slice compute (projections + rope + vT), as a fill generator ----
        PAIRS = ((0, 1, "q"), (2, 3, "k"), (4, 5, "v"))

        def slice_steps(g, xs, cos, sin, qts, pairs=PAIRS, yield_every=1):
            pend = []  # deferred post-steps, run with >=2 d-steps of lag

            def rope_step(kind, hh, t_in):
                def run():
                    pps = pshared.tile([128, 512], F32, tag="sh", bufs=3,
                                       name="pps")
                    nc.tensor.matmul(pps[:], PT_b, t_in[:],
                                     start=True, stop=True,
                                     skip_group_check=True)
                    t1 = work.tile([128, 512], BF16, tag="t1", bufs=2,
                                   name="t1")
                    nc.gpsimd.tensor_mul(t1[:], t_in[:], cos[:])
                    t2 = work.tile([128, 512], BF16, tag="t2", bufs=2,
                                   name="t2")
                    nc.vector.tensor_mul(t2[:], pps[:], sin[:])
                    if kind == "q":
                        nc.gpsimd.tensor_add(qts[hh][:], t1[:], t2[:])
                    else:
                        nc.gpsimd.tensor_add(
                            kT[hh][:, g * 512:(g + 1) * 512], t1[:], t2[:])
                return run

            def vt_step(hh, vtmp):
                def run():
                    tp = pshared.tile([128, 512], BF16, tag="sh", bufs=3,
                                      name="vtp")
                    for t in range(4):
                        nc.tensor.matmul(
                            tp[:, t * 128:(t + 1) * 128],
                            vtmp[:, t * 128:(t + 1) * 128],
                            ident_b,
                            is_transpose=True, skip_group_check=True,
                        )
                    dst = v_sb[:].rearrange("p (b x) -> p b x", x=256)[
                        :, 4 * g:4 * g + 4, hh * 128:(hh + 1) * 128]
                    src = tp[:].rearrange("p (b x) -> p b x", x=128)
                    nc.vector.tensor_copy(dst, src)
                return run

            for (jta, jtb, kind) in pairs:
                acc = [pacc.tile([128, 512], F32, tag="acc", bufs=3, name="acc")
                       for _ in range(2)]
                for d in range(ND):
                    xr = xs[d // 2][:, (d % 2) * 512:(d % 2) * 512 + 512]
                    for i, jt in enumerate((jta, jtb)):
                        nc.tensor.matmul(
                            acc[i][:],
                            w_r[:, d * 768 + jt * 128:d * 768 + (jt + 1) * 128],
                            xr,
                            start=(d == 0), stop=(d == ND - 1),
                            skip_group_check=True,
                        )
                    if d % yield_every == yield_every - 1:
                        yield
                # drain the accs to SBUF (Pool + DVE in parallel); defer the
                # compute halves and give the drains two sweep-blocks of lead
                for hh in range(2):
                    t_in = work.tile([128, 512], BF16,
                                     tag="vtmp" if kind == "v" else "tin",
                                     bufs=4, name="tin")
                    nc.scalar.copy(t_in[:, 0:256], acc[hh][:, 0:256])
                    nc.vector.tensor_copy(t_in[:, 256:512],
                                          acc[hh][:, 256:512])
                    pend.append(vt_step(hh, t_in) if kind == "v"
                                else rope_step(kind, hh, t_in))
                # run the deferred steps at the pair boundary: they separate
                # this pair's acc drains from the next pair's bank reuse
                yield
                while pend:
                    pend.pop(0)()
                    yield
            while pend:
                pend.pop(0)()
                yield

        # ---- group tail (recip/bc/ot) + output projection, fill generators --
        def tail_steps(g, pvz):
            ots = []
            for hh in range(2):
                if hh == 0:
                    pv, rc = pvz[0]  # recip already done inline in sweep
                else:
                    pv, master = pvz[1]
                    zz = pshared.tile([1, 512], F32, tag="sh", bufs=3,
                                      name="z")
                    nc.tensor.matmul(zz[:], onescol_r, master[:],
                                     start=True, stop=True,
                                     skip_group_check=True)
                    rc = work.tile([1, 512], F32R, tag="rc", bufs=2, name="rc")
                    with nc.allow_low_precision(reason="fp32r rounding of 1/Z"):
                        nc.vector.reciprocal(rc[:], zz[:])
                bc = pshared.tile([128, 512], F32, tag="sh", bufs=3, name="bc")
                nc.tensor.matmul(bc[:], onesrow_r, rc[:], start=True, stop=True,
                                 skip_group_check=True)
                bcs = work.tile([128, 512], BF16, tag="bcs", bufs=2, name="bcs")
                nc.scalar.copy(bcs[:], bc[:])
                ot = work.tile([128, 512], BF16, tag=f"ot{hh}", bufs=2,
                               name=f"ot{hh}")
                nc.vector.tensor_mul(ot[:], pv[:], bcs[:])
                ots.append(ot)
                yield
            pvz.append(ots)  # hand ots to op_steps via shared list

        def op_steps(g, pvz):
            last = g == NG - 1
            while len(pvz) < 3:
                yield  # wait until tail_steps appended ots (same deque order)
            ots = pvz[2]
            for t in range(4):
                osb = work.tile([128, D], BF16, tag="osb", bufs=3, name="osb")
                for n in range(4):
                    op = pshared.tile([128, 512], F32, tag="sh", bufs=3,
                                      name="op")
                    for hh in range(2):
                        nc.tensor.matmul(
                            op[:],
                            ots[hh][:, t * 128:(t + 1) * 128],
                            wo_r[:, hh * D + n * 512:hh * D + (n + 1) * 512],
                            start=(hh == 0), stop=(hh == 1),
                            skip_group_check=True,
                        )
                    if last and n % 2 == 1:
                        nc.scalar.copy(osb[:, n * 512:(n + 1) * 512], op[:])
                    else:
                        nc.vector.tensor_copy(osb[:, n * 512:(n + 1) * 512],
                                              op[:])
                    yield
                nc.sync.dma_start(
                    out_d.ap()[g * 512 + t * 128:g * 512 + (t + 1) * 128, :],
                    osb[:],
                )

        # ---- fill machinery ----
        fill_q = deque()

        def pull_fill(n=1):
            for _ in range(n):
                while fill_q:
                    try:
                        next(fill_q[0])
                        break
                    except StopIteration:
                        fill_q.popleft()
                else:
                    return

        def drain_fill():
            while fill_q:
                pull_fill()

        # ---- attention sweep for group g (emits blocks, pulls fill) ----
        # Block order: both heads' non-diagonal blocks first (h0 then h1),
        # then the diagonal blocks of h0 and h1.  This pushes the kT/v_sb
        # dependency on slice g to the END of the sweep, so slice g's k/v
        # projection passes can serve as fill for sweep g itself (used for
        # the last group, whose sweep otherwise has no projection fill).
        def sweep(g, qts):
            nkb = 4 * g + 4
            pvz_out = []
            pv = [None, None]
            master = [None, None]
            quad = [[], []]
            nblk = [0, 0]

            def madd(hh, x_ap, o, w):
                if master[hh] is None:
                    assert o == 0 and w == 512
                    master[hh] = work.tile([128, 512], F32R, tag=f"m{hh}",
                                           bufs=2, name=f"m{hh}")
                    nc.vector.tensor_copy(master[hh][:], x_ap)
                elif o == 0 and w == 512:
                    nc.vector.tensor_add(master[hh][:], master[hh][:], x_ap)
                else:
                    nc.vector.tensor_add(master[hh][:, o:o + w],
                                         master[hh][:, o:o + w], x_ap)

            order = [(hh, j) for hh in range(2) for j in range(4 * g)] + \
                    [(hh, 4 * g + p) for hh in range(2) for p in range(4)]
            for (hh, j) in order:
                if pv[hh] is None:
                    pv[hh] = ppv.tile([128, 512], F32, tag="pv", bufs=2,
                                      name=f"pv{hh}")
                p = j - 4 * g
                diag = p >= 0
                o, w = (DO[p], DW[p]) if diag else (0, 512)
                sim = pshared.tile([128, 512], F32, tag="sh", bufs=3,
                                   name="sim")
                nc.tensor.matmul(
                    sim[:, 0:w],
                    kT[hh][:, j * 128:(j + 1) * 128],
                    qts[hh][:, o:512],
                    start=True, stop=True, skip_group_check=True,
                )
                pr = work.tile([128, 512], BF16, tag="pr", bufs=8,
                               name="pr")
                nc.scalar.activation(pr[:, 0:w], sim[:, 0:w], Exp,
                                     scale=SCALE)
                if diag:
                    nc.vector.tensor_mul(pr[:, 0:w], pr[:, 0:w], masks[p])
                nc.tensor.matmul(
                    pv[hh][:, o:512],
                    v_sb[:, j * 256 + hh * 128:j * 256 + hh * 128 + 128],
                    pr[:, 0:w],
                    start=(nblk[hh] == 0), stop=(nblk[hh] == nkb - 1),
                    skip_group_check=True,
                )
                nblk[hh] += 1
                # Z fold
                if not diag:
                    q = quad[hh]
                    q.append(pr)
                    if len(q) == 4:
                        s1 = work.tile([128, 512], BF16, tag="zf", bufs=4,
                                       name="zf")
                        nc.gpsimd.tensor_add(s1[:], q[0][:], q[1][:])
                        s2 = work.tile([128, 512], BF16, tag="zf", bufs=4,
                                       name="zf")
                        nc.vector.tensor_add(s2[:], s1[:], q[2][:])
                        if master[hh] is None:
                            master[hh] = work.tile([128, 512], F32R,
                                                   tag=f"m{hh}", bufs=2,
                                                   name=f"m{hh}")
                            nc.vector.tensor_add(master[hh][:], s2[:],
                                                 q[3][:])
                        else:
                            s3 = work.tile([128, 512], BF16, tag="zf",
                                           bufs=4, name="zf")
                            nc.vector.tensor_add(s3[:], s2[:], q[3][:])
                            nc.gpsimd.tensor_add(master[hh][:],
                                                 master[hh][:], s3[:])
                        quad[hh] = []
                else:
                    madd(hh, pr[:, 0:w], o, w)
                pull_fill(1)
                yield
                if nblk[0] == nkb and master[0] is not None and \
                        len(pvz_out) == 0:
                    # h0 finished all its blocks: emit its Z + recip now
                    pull_fill(3)
                    zz = pshared.tile([1, 512], F32, tag="sh", bufs=3,
                                      name="z")
                    nc.tensor.matmul(zz[:], onescol_r, master[0][:],
                                     start=True, stop=True,
                                     skip_group_check=True)
                    rc = work.tile([1, 512], F32R, tag="rc", bufs=2, name="rc")
                    with nc.allow_low_precision(reason="fp32r rounding of 1/Z"):
                        nc.vector.reciprocal(rc[:], zz[:])
                    pvz_out.append((pv[0], rc))
            assert not quad[0] and not quad[1]
            # h1's Z is emitted by tail_steps after h0's bc, hiding the
            # fold-chain tail behind h0's normalization
            pvz_out.append((pv[1], master[1]))
            return pvz_out

        # ================= main program =================
        xs0, cos0, sin0 = issue_x(0, interleave_w=True)
        nc.sync.dma_start(wo_r[:, 0:D], wo_d.ap()[0:128, :])
        nc.sync.dma_start(wo_r[:, D:2 * D], wo_d.ap()[128:256, :])

        qts_all = {}

        def new_qts():
            return [work.tile([128, 512], BF16, tag=f"qt{h}", bufs=2,
                              name=f"qt{h}") for h in range(2)]

        # slice 0 runs un-filled (nothing to overlap with yet)
        qts_all[0] = new_qts()
        for _ in slice_steps(0, xs0, cos0, sin0, qts_all[0]):
            pass

        for g in range(NG):
            if g + 1 < NG:
                xs_n, cos_n, sin_n = issue_x(g + 1)
                qts_all[g + 1] = new_qts()
                if g + 1 == NG - 1:
                    # last slice: only the q projection must precede sweep 7;
                    # its k/v passes become fill INSIDE sweep 7 (whose
                    # diagonal blocks, the only consumers, come last)
                    fill_q.append(slice_steps(g + 1, xs_n, cos_n, sin_n,
                                              qts_all[g + 1],
                                              pairs=PAIRS[:1]))
                    kv7 = slice_steps(g + 1, xs_n, cos_n, sin_n,
                                      qts_all[g + 1], pairs=PAIRS[1:],
                                      yield_every=2)
                else:
                    fill_q.append(
                        slice_steps(g + 1, xs_n, cos_n, sin_n,
                                    qts_all[g + 1]))
            # run the sweep (pulls fill: [tail g-1, op g-1, proj g+1])
            sw = sweep(g, qts_all[g])
            pvz = None
            try:
                while True:
                    next(sw)
            except StopIteration as e:
                pvz = e.value
            # everything queued must land before the next sweep's sims
            drain_fill()
            fill_q.append(tail_steps(g, pvz))
            fill_q.append(op_steps(g, pvz))
            if g == NG - 1:
                drain_fill()
            elif g + 1 == NG - 1:
                fill_q.append(kv7)

    nc.compile()
    return nc


def _host_tables(S: int):
    inv = 1.0 / (ROPE_BASE ** (np.arange(0, DH, 2, dtype=np.float64) / DH))
    t = np.arange(S, dtype=np.float64)
    fr = np.outer(t, inv)  # [S, 64]
    cos = np.repeat(np.cos(fr), 2, axis=1)
    sin = np.repeat(np.sin(fr), 2, axis=1)
    cs = np.concatenate([cos.T, sin.T], axis=1).astype(BF)  # [128, 2S]

    PT = np.zeros((DH, DH), np.float32)
    for m in range(DH // 2):
        PT[2 * m + 1, 2 * m] = -1.0
        PT[2 * m, 2 * m + 1] = 1.0
    cb = np.zeros((128, 2304), np.float32)
    cb[:, 0:128] = PT
    cb[:, 128:256] = np.eye(128, dtype=np.float32)
    # diagonal-block causal masks: tile col c (query 128p+c) vs key partition
    part = np.arange(128)[:, None]
    for p in range(4):
        w = (512, 384, 256, 128)[p]
        c = np.arange(w)[None, :]
        cb[:, 256 + p * 512:256 + p * 512 + w] = (c >= part).astype(np.float32)
    cb = cb.astype(BF)

    cr = np.zeros((128, 130), np.float32)
    cr[:, 0] = 1.0        # onescol
    cr[0, 1:129] = 1.0    # onesrow
    return cs, cb, cr


def kernel(x, mask, wq, wk, wv, wo):
    x = np.asarray(x, dtype=np.float32)
    wq = np.asarray(wq, dtype=np.float32)
    wk = np.asarray(wk, dtype=np.float32)
    wv = np.asarray(wv, dtype=np.float32)
    wo = np.asarray(wo, dtype=np.float32)
    S = x.shape[0]

    if S not in _BUILD_CACHE:
        _BUILD_CACHE[S] = _build(S)
    nc = _BUILD_CACHE[S]

    cs, cb, cr = _host_tables(S)
    xT = np.ascontiguousarray(x.T.astype(BF))

    in_maps = []
    for c in range(NCORES):
        hsl = slice(c * HPC * DH, (c + 1) * HPC * DH)
        wqT = wq[hsl].T.reshape(D, 2, DH)
        wkT = wk[hsl].T.reshape(D, 2, DH)
        wvT = wv[hsl].T.reshape(D, 2, DH)
        wqkvT = np.concatenate(
            [wqT[:, 0], wqT[:, 1], wkT[:, 0], wkT[:, 1], wvT[:, 0], wvT[:, 1]],
            axis=1,
        ).astype(BF)
        woT = np.ascontiguousarray(wo[:, hsl].T.astype(BF))
        in_maps.append(
            {
                "xT": xT,
                "wqkvT": np.ascontiguousarray(wqkvT),
                "woT": woT,
                "cs": cs,
                "cb": cb,
                "cr": cr,
            }
        )

    res = run_bass_kernel_spmd(
        nc, in_maps, core_ids=list(range(NCORES)), trace=TRACE
    )
    global LAST_RESULT
    LAST_RESULT = res
    out = np.zeros((S, D), np.float32)
    for r in res.results:
        out += np.asarray(r["outp"], dtype=np.float32)
    return out


# revision 50
# speedup vs baseline: 1.0473x; 1.0473x over previous
"""Trainium2 Bass kernel for nn_Attention_48000554500172.

16-head causal attention with RoPE (S=4096, D=2048, H=16, DH=128), sharded
over heads across 8 NeuronCores (2 heads/core, tensor parallel). Each core
computes its heads' QKV projections, RoPE, causal softmax attention and the
partial output projection; the 8 partial [S, D] outputs are summed on host
(the all-reduce of the sharding hint).

v2 design (vs the 452us fp32r baseline):
- All matmul inputs in bf16 (1 cycle/row on the PE at ANY moving size, same
  as fp32r>=256, but: half the DMA bytes, 2-4x DVE element ops, and no
  min-256 moving-width constraint so diagonal blocks shrink to their true
  causal widths). PSUM accumulation stays fp32; measured rel err ~1e-2/2
  budget.
- No fp32->fp32r rounding copies: DMA lands bf16 directly.
- Softmax denominators: instead of ~88 ones-matmuls per head (each costing
  a full 512-row pass on the PE), probs tiles are folded on the DVE (bf16
  quad trees + one f32r running master per head/group, in-place partial
  width adds for the diagonal) and ONE ones-matmul per (head, group)
  computes the column sums. PE cost for Z drops ~30us.
- Phase interleaving: the per-block attention pipeline is exp-paced on the
  ACT engine (612ns/block vs 426ns of PE sim+PV work), so the projection
  matmuls of slice g+1 and the output projection of group g-1 are emitted
  as "fill" between the sweep blocks of group g. The PE never idles waiting
  for exp.
- PSUM: pacc(2: proj jt-pair accumulators) + pshared(3: sim/rope-P/vT/
  bc/outproj rotation) + ppvz(3: pv_h0, z_h0, pv_h1, z_h1 ring) = 8 banks,
  sized so consecutive groups/heads never serialize on banks.
- Startup: slice-0 x chunks and weight chunks DMA'd interleaved in exactly
  d-loop consumption order; first matmul at ~1.5us.
"""
import math
import numpy as np
import ml_dtypes
from collections import deque
from contextlib import ExitStack

import concourse.bass as bass
import concourse.tile as tile
from concourse import bacc, mybir
from concourse.bass_utils import run_bass_kernel_spmd

D, H, DH = 2048, 16, 128
NCORES = 8
HPC = H // NCORES  # 2 heads per core
ROPE_BASE = 10000.0
SCALE = 1.0 / math.sqrt(DH)
F32 = mybir.dt.float32
F32R = mybir.dt.float32r
BF16 = mybir.dt.bfloat16
Exp = mybir.ActivationFunctionType.Exp
BF = ml_dtypes.bfloat16

_BUILD_CACHE: dict = {}
TRACE = False
LAST_RESULT = None

# diagonal key-block p: query cols [128p, 512) are unmasked
DW = (512, 384, 256, 128)
DO = (0, 128, 256, 384)


def _build(S: int):
    assert S % 512 == 0
    NG = S // 512   # 8 query groups == s-slices
    ND = D // 128   # 16 contraction tiles
    NB = S // 128   # 32 key blocks

    nc = bacc.Bacc("TRN2", target_bir_lowering=False, debug=False)

    xT_d = nc.dram_tensor("xT", [D, S], BF16, kind="ExternalInput")
    w_d = nc.dram_tensor("wqkvT", [D, 768], BF16, kind="ExternalInput")
    wo_d = nc.dram_tensor("woT", [2 * DH, D], BF16, kind="ExternalInput")
    cs_d = nc.dram_tensor("cs", [128, 2 * S], BF16, kind="ExternalInput")
    cb_d = nc.dram_tensor("cb", [128, 2304], BF16, kind="ExternalInput")
    cr_d = nc.dram_tensor("cr", [128, 130], F32R, kind="ExternalInput")
    out_d = nc.dram_tensor("outp", [S, D], BF16, kind="ExternalOutput")

    with tile.TileContext(nc) as tc, ExitStack() as ctx:
        persist = ctx.enter_context(tc.tile_pool(name="persist", bufs=1))
        work = ctx.enter_context(tc.tile_pool(name="work", bufs=2))
        pacc = ctx.enter_context(tc.tile_pool(name="pacc", bufs=3, space="PSUM"))
        pshared = ctx.enter_context(tc.tile_pool(name="psh", bufs=3, space="PSUM"))
        ppv = ctx.enter_context(tc.tile_pool(name="ppv", bufs=2, space="PSUM"))

        # ---- persistent tiles ----
        w_r = persist.tile([128, ND * 768], BF16, tag="w", name="wr")
        wo_r = persist.tile([128, 2 * D], BF16, tag="wo", name="wor")
        kT = [persist.tile([128, S], BF16, tag=f"kT{h}", name=f"kT{h}")
              for h in range(2)]
        v_sb = persist.tile([128, NB * 256], BF16, tag="v", name="vsb")
        cb_t = persist.tile([128, 2304], BF16, tag="cb", name="cbt")
        cr_t = persist.tile([128, 130], F32R, tag="cr", name="crt")
        PT_b = cb_t[:, 0:128]
        ident_b = cb_t[:, 128:256]
        onescol_r = cr_t[:, 0:1]
        onesrow_r = cr_t[0:1, 1:129]
        masks = [cb_t[:, 256 + p * 512:256 + p * 512 + DW[p]] for p in range(4)]

        # ---- DMA issue helpers ----
        def issue_x(g, interleave_w=False):
            """DMA x chunks (+cos/sin) for slice g; optionally interleave the
            16 w chunks + cb in d-consumption order (startup)."""
            xs = []
            for dd in range(ND // 2):
                t = work.tile([128, 1024], BF16, tag="xs", bufs=18, name="xs")
                src = xT_d.ap()[dd * 256:(dd + 1) * 256, g * 512:(g + 1) * 512]
                nc.sync.dma_start(
                    t[:].rearrange("b (a c) -> b a c", a=2),
                    src.rearrange("(a b) c -> b a c", a=2),
                )
                xs.append(t)
                if interleave_w:
                    if dd == 0:
                        # singles so the very first matmul unblocks sooner
                        for d in (0, 1):
                            nc.sync.dma_start(
                                w_r[:, d * 768:(d + 1) * 768],
                                w_d.ap()[d * 128:(d + 1) * 128, :],
                            )
                    else:
                        # two w chunks in one strided DMA, consumption order
                        wdst = w_r[:, dd * 1536:(dd + 1) * 1536]
                        wsrc = w_d.ap()[dd * 256:(dd + 1) * 256, :]
                        nc.sync.dma_start(
                            wdst.rearrange("b (a c) -> b a c", a=2),
                            wsrc.rearrange("(a b) c -> b a c", a=2),
                        )
                    if dd == 1:
                        nc.sync.dma_start(cb_t[:], cb_d.ap())
                    if dd == 2:
                        nc.sync.dma_start(cr_t[:], cr_d.ap())
            cos = work.tile([128, 512], BF16, tag="cos", bufs=2, name="cos")
            sin = work.tile([128, 512], BF16, tag="sin", bufs=2, name="sin")
            nc.sync.dma_start(cos[:], cs_d.ap()[:, g * 512:(g + 1) * 512])
            nc.sync.dma_start(sin[:], cs_d.ap()[:, S + g * 512:S + (g + 1) * 512])
            return xs, cos, sin

        # ---- slice compute (projections + rope + vT), as a fill generator ----
        PAIRS = ((0, 1, "q"), (2, 3, "k"), (4, 5, "v"))

        def slice_steps(g, xs, cos, sin, qts, pairs=PAIRS, yield_every=1):
            pend = []  # deferred post-steps, run with >=2 d-steps of lag

            def rope_step(kind, hh, t_in):
                def run():
                    pps = pshared.tile([128, 512], F32, tag="sh", bufs=3,
                                       name="pps")
                    nc.tensor.matmul(pps[:], PT_b, t_in[:],
                                     start=True, stop=True,
                                     skip_group_check=True)
                    t1 = work.tile([128, 512], BF16, tag="t1", bufs=2,
                                   name="t1")
                    nc.gpsimd.tensor_mul(t1[:], t_in[:], cos[:])
                    t2 = work.tile([128, 512], BF16, tag="t2", bufs=2,
                                   name="t2")
                    nc.vector.tensor_mul(t2[:], pps[:], sin[:])
                    if kind == "q":
                        nc.gpsimd.tensor_add(qts[hh][:], t1[:], t2[:])
                    else:
                        nc.gpsimd.tensor_add(
                            kT[hh][:, g * 512:(g + 1) * 512], t1[:], t2[:])
                return run

            def vt_step(hh, vtmp):
                def run():
                    tp = pshared.tile([128, 512], BF16, tag="sh", bufs=3,
                                      name="vtp")
                    for t in range(4):
                        nc.tensor.matmul(
                            tp[:, t * 128:(t + 1) * 128],
                            vtmp[:, t * 128:(t + 1) * 128],
                            ident_b,
                            is_transpose=True, skip_group_check=True,
                        )
                    dst = v_sb[:].rearrange("p (b x) -> p b x", x=256)[
                        :, 4 * g:4 * g + 4, hh * 128:(hh + 1) * 128]
                    src = tp[:].rearrange("p (b x) -> p b x", x=128)
                    nc.vector.tensor_copy(dst, src)
                return run

            for (jta, jtb, kind) in pairs:
                acc = [pacc.tile([128, 512], F32, tag="acc", bufs=3, name="acc")
                       for _ in range(2)]
                for d in range(ND):
                    xr = xs[d // 2][:, (d % 2) * 512:(d % 2) * 512 + 512]
                    for i, jt in enumerate((jta, jtb)):
                        nc.tensor.matmul(
                            acc[i][:],
                            w_r[:, d * 768 + jt * 128:d * 768 + (jt + 1) * 128],
                            xr,
                            start=(d == 0), stop=(d == ND - 1),
                            skip_group_check=True,
                        )
                    if d % yield_every == yield_every - 1:
                        yield
                # drain the accs to SBUF (Pool + DVE in parallel); defer the
                # compute halves and give the drains two sweep-blocks of lead
                for hh in range(2):
                    t_in = work.tile([128, 512], BF16,
                                     tag="vtmp" if kind == "v" else "tin",
                                     bufs=4, name="tin")
                    nc.scalar.copy(t_in[:, 0:256], acc[hh][:, 0:256])
                    nc.vector.tensor_copy(t_in[:, 256:512],
                                          acc[hh][:, 256:512])
                    pend.append(vt_step(hh, t_in) if kind == "v"
                                else rope_step(kind, hh, t_in))
                # run the deferred steps at the pair boundary: they separate
                # this pair's acc drains from the next pair's bank reuse
                yield
                while pend:
                    pend.pop(0)()
                    yield
            while pend:
                pend.pop(0)()
                yield

        # ---- group tail (recip/bc/ot) + output projection, fill generators --
        def tail_steps(g, pvz):
            ots = []
            for hh in range(2):
                if hh == 0:
                    pv, rc = pvz[0]  # recip already done inline in sweep
                else:
                    pv, master = pvz[1]
                    zz = pshared.tile([1, 512], F32, tag="sh", bufs=3,
                                      name="z")
                    nc.tensor.matmul(zz[:], onescol_r, master[:],
                                     start=True, stop=True,
                                     skip_group_check=True)
                    rc = work.tile([1, 512], F32R, tag="rc", bufs=2, name="rc")
                    with nc.allow_low_precision(reason="fp32r rounding of 1/Z"):
                        nc.vector.reciprocal(rc[:], zz[:])
                bc = pshared.tile([128, 512], F32, tag="sh", bufs=3, name="bc")
                nc.tensor.matmul(bc[:], onesrow_r, rc[:], start=True, stop=True,
                                 skip_group_check=True)
                bcs = work.tile([128, 512], BF16, tag="bcs", bufs=2, name="bcs")
                nc.scalar.copy(bcs[:], bc[:])
                ot = work.tile([128, 512], BF16, tag=f"ot{hh}", bufs=2,
                               name=f"ot{hh}")
                nc.vector.tensor_mul(ot[:], pv[:], bcs[:])
                ots.append(ot)
                yield
            pvz.append(ots)  # hand ots to op_steps via shared list

        def op_steps(g, pvz):
            last = g == NG - 1
            while len(pvz) < 3:
                yield  # wait until tail_steps appended ots (same deque order)
            ots = pvz[2]
            for t in range(4):
                osb = work.tile([128, D], BF16, tag="osb", bufs=3, name="osb")
                for n in range(4):
                    op = pshared.tile([128, 512], F32, tag="sh", bufs=3,
                                      name="op")
                    for hh in range(2):
                        nc.tensor.matmul(
                            op[:],
                            ots[hh][:, t * 128:(t + 1) * 128],
                            wo_r[:, hh * D + n * 512:hh * D + (n + 1) * 512],
                            start=(hh == 0), stop=(hh == 1),
                            skip_group_check=True,
                        )
                    if last and n % 2 == 1:
                        nc.scalar.copy(osb[:, n * 512:(n + 1) * 512], op[:])
                    else:
                        nc.vector.tensor_copy(osb[:, n * 512:(n + 1) * 512],
                                              op[:])
                    yield
                nc.sync.dma_start(
                    out_d.ap()[g * 512 + t * 128:g * 512 + (t + 1) * 128, :],
                    osb[:],
                )

        # ---- fill machinery ----
        fill_q = deque()

        def pull_fill(n=1):
            for _ in range(n):
                while fill_q:
                    try:
                        next(fill_q[0])
                        break
                    except StopIteration:
                        fill_q.popleft()
                else:
                    return

        def drain_fill():
            while fill_q:
                pull_fill()

        # ---- attention sweep for group g (emits blocks, pulls fill) ----
        # Block order: both heads' non-diagonal blocks first (h0 then h1),
        # then the diagonal blocks of h0 and h1.  This pushes the kT/v_sb
        # dependency on slice g to the END of the sweep, so slice g's k/v
        # projection passes can serve as fill for sweep g itself (used for
        # the last group, whose sweep otherwise has no projection fill).
        def sweep(g, qts):
            nkb = 4 * g + 4
            pvz_out = []
            pv = [None, None]
            master = [None, None]
            quad = [[], []]
            nblk = [0, 0]

            def madd(hh, x_ap, o, w):
                if master[hh] is None:
                    assert o == 0 and w == 512
                    master[hh] = work.tile([128, 512], F32R, tag=f"m{hh}",
                                           bufs=2, name=f"m{hh}")
                    nc.vector.tensor_copy(master[hh][:], x_ap)
                elif o == 0 and w == 512:
                    nc.vector.tensor_add(master[hh][:], master[hh][:], x_ap)
                else:
                    nc.vector.tensor_add(master[hh][:, o:o + w],
                                         master[hh][:, o:o + w], x_ap)

            order = [(hh, j) for hh in range(2) for j in range(4 * g)] + \
                    [(hh, 4 * g + p) for hh in range(2) for p in range(4)]
            for (hh, j) in order:
                if pv[hh] is None:
                    pv[hh] = ppv.tile([128, 512], F32, tag="pv", bufs=2,
                                      name=f"pv{hh}")
                p = j - 4 * g
                diag = p >= 0
                o, w = (DO[p], DW[p]) if diag else (0, 512)
                sim = pshared.tile([128, 512], F32, tag="sh", bufs=3,
                                   name="sim")
                nc.tensor.matmul(
                    sim[:, 0:w],
                    kT[hh][:, j * 128:(j + 1) * 128],
                    qts[hh][:, o:512],
                    start=True, stop=True, skip_group_check=True,
                )
                pr = work.tile([128, 512], BF16, tag="pr", bufs=8,
                               name="pr")
                nc.scalar.activation(pr[:, 0:w], sim[:, 0:w], Exp,
                                     scale=SCALE)
                if diag:
                    nc.vector.tensor_mul(pr[:, 0:w], pr[:, 0:w], masks[p])
                nc.tensor.matmul(
                    pv[hh][:, o:512],
                    v_sb[:, j * 256 + hh * 128:j * 256 + hh * 128 + 128],
                    pr[:, 0:w],
                    start=(nblk[hh] == 0), stop=(nblk[hh] == nkb - 1),
                    skip_group_check=True,
                )
                nblk[hh] += 1
                # Z fold
                if not diag:
                    q = quad[hh]
                    q.append(pr)
                    if len(q) == 4:
                        s1 = work.tile([128, 512], BF16, tag="zf", bufs=4,
                                       name="zf")
                        nc.gpsimd.tensor_add(s1[:], q[0][:], q[1][:])
                        s2 = work.tile([128, 512], BF16, tag="zf", bufs=4,
                                       name="zf")
                        nc.vector.tensor_add(s2[:], s1[:], q[2][:])
                        if master[hh] is None:
                            master[hh] = work.tile([128, 512], F32R,
                                                   tag=f"m{hh}", bufs=2,
                                                   name=f"m{hh}")
                            nc.vector.tensor_add(master[hh][:], s2[:],
                                                 q[3][:])
                        else:
                            s3 = work.tile([128, 512], BF16, tag="zf",
                                           bufs=4, name="zf")
                            nc.vector.tensor_add(s3[:], s2[:], q[3][:])
                            nc.vector.tensor_add(master[hh][:],
                                                 master[hh][:], s3[:])
                        quad[hh] = []
                else:
                    madd(hh, pr[:, 0:w], o, w)
                pull_fill(1)
                yield
                if nblk[0] == nkb and master[0] is not None and \
                        len(pvz_out) == 0:
                    # h0 finished all its blocks: emit its Z + recip now
                    pull_fill(3)
                    zz = pshared.tile([1, 512], F32, tag="sh", bufs=3,
                                      name="z")
                    nc.tensor.matmul(zz[:], onescol_r, master[0][:],
                                     start=True, stop=True,
                                     skip_group_check=True)
                    rc = work.tile([1, 512], F32R, tag="rc", bufs=2, name="rc")
                    with nc.allow_low_precision(reason="fp32r rounding of 1/Z"):
                        nc.vector.reciprocal(rc[:], zz[:])
                    pvz_out.append((pv[0], rc))
            assert not quad[0] and not quad[1]
            # h1's Z is emitted by tail_steps after h0's bc, hiding the
            # fold-chain tail behind h0's normalization
            pvz_out.append((pv[1], master[1]))
            return pvz_out

        # ================= main program =================
        xs0, cos0, sin0 = issue_x(0, interleave_w=True)
        nc.sync.dma_start(wo_r[:, 0:D], wo_d.ap()[0:128, :])
        nc.sync.dma_start(wo_r[:, D:2 * D], wo_d.ap()[128:256, :])

        qts_all = {}

        def new_qts():
            return [work.tile([128, 512], BF16, tag=f"qt{h}", bufs=2,
                              name=f"qt{h}") for h in range(2)]

        # slice 0 runs un-filled (nothing to overlap with yet)
        qts_all[0] = new_qts()
        for _ in slice_steps(0, xs0, cos0, sin0, qts_all[0]):
            pass

        for g in range(NG):
            if g + 1 < NG:
                xs_n, cos_n, sin_n = issue_x(g + 1)
                qts_all[g + 1] = new_qts()
                if g + 1 == NG - 1:
                    # last slice: only the q projection must precede sweep 7;
                    # its k/v passes become fill INSIDE sweep 7 (whose
                    # diagonal blocks, the only consumers, come last)
                    fill_q.append(slice_steps(g + 1, xs_n, cos_n, sin_n,
                                              qts_all[g + 1],
                                              pairs=PAIRS[:1]))
                    kv7 = slice_steps(g + 1, xs_n, cos_n, sin_n,
                                      qts_all[g + 1], pairs=PAIRS[1:],
                                      yield_every=2)
                else:
                    fill_q.append(
                        slice_steps(g + 1, xs_n, cos_n, sin_n,
                                    qts_all[g + 1]))
            # run the sweep (pulls fill: [tail g-1, op g-1, proj g+1])
            sw = sweep(g, qts_all[g])
            pvz = None
            try:
                while True:
                    next(sw)
            except StopIteration as e:
                pvz = e.value
            # everything queued must land before the next sweep's sims
            drain_fill()
            fill_q.append(tail_steps(g, pvz))
            fill_q.append(op_steps(g, pvz))
            if g == NG - 1:
                drain_fill()
            elif g + 1 == NG - 1:
                fill_q.append(kv7)

    nc.compile()
    return nc


def _host_tables(S: int):
    inv = 1.0 / (ROPE_BASE ** (np.arange(0, DH, 2, dtype=np.float64) / DH))
    t = np.arange(S, dtype=np.float64)
    fr = np.outer(t, inv)  # [S, 64]
    cos = np.repeat(np.cos(fr), 2, axis=1)
    sin = np.repeat(np.sin(fr), 2, axis=1)
    cs = np.concatenate([cos.T, sin.T], axis=1).astype(BF)  # [128, 2S]

    PT = np.zeros((DH, DH), np.float32)
    for m in range(DH // 2):
        PT[2 * m + 1, 2 * m] = -1.0
        PT[2 * m, 2 * m + 1] = 1.0
    cb = np.zeros((128, 2304), np.float32)
    cb[:, 0:128] = PT
    cb[:, 128:256] = np.eye(128, dtype=np.float32)
    # diagonal-block causal masks: tile col c (query 128p+c) vs key partition
    part = np.arange(128)[:, None]
    for p in range(4):
        w = (512, 384, 256, 128)[p]
        c = np.arange(w)[None, :]
        cb[:, 256 + p * 512:256 + p * 512 + w] = (c >= part).astype(np.float32)
    cb = cb.astype(BF)

    cr = np.zeros((128, 130), np.float32)
    cr[:, 0] = 1.0        # onescol
    cr[0, 1:129] = 1.0    # onesrow
    return cs, cb, cr


def kernel(x, mask, wq, wk, wv, wo):
    x = np.asarray(x, dtype=np.float32)
    wq = np.asarray(wq, dtype=np.float32)
    wk = np.asarray(wk, dtype=np.float32)
    wv = np.asarray(wv, dtype=np.float32)
    wo = np.asarray(wo, dtype=np.float32)
    S = x.shape[0]

    if S not in _BUILD_CACHE:
        _BUILD_CACHE[S] = _build(S)
    nc = _BUILD_CACHE[S]

    cs, cb, cr = _host_tables(S)
    xT = np.ascontiguousarray(x.T.astype(BF))

    in_maps = []
    for c in range(NCORES):
        hsl = slice(c * HPC * DH, (c + 1) * HPC * DH)
        wqT = wq[hsl].T.reshape(D, 2, DH)
        wkT = wk[hsl].T.reshape(D, 2, DH)
        wvT = wv[hsl].T.reshape(D, 2, DH)
        wqkvT = np.concatenate(
            [wqT[:, 0], wqT[:, 1], wkT[:, 0], wkT[:, 1], wvT[:, 0], wvT[:, 1]],
            axis=1,
        ).astype(BF)
        woT = np.ascontiguousarray(wo[:, hsl].T.astype(BF))
        in_maps.append(
            {
                "xT": xT,
                "wqkvT": np.ascontiguousarray(wqkvT),
                "woT": woT,
                "cs": cs,
                "cb": cb,
                "cr": cr,
            }
        )

    res = run_bass_kernel_spmd(
        nc, in_maps, core_ids=list(range(NCORES)), trace=TRACE
    )
    global LAST_RESULT
    LAST_RESULT = res
    out = np.zeros((S, D), np.float32)
    for r in res.results:
        out += np.asarray(r["outp"], dtype=np.float32)
    return out


# revision 51
# speedup vs baseline: 1.0622x; 1.0142x over previous
"""Trainium2 Bass kernel for nn_Attention_48000554500172.

16-head causal attention with RoPE (S=4096, D=2048, H=16, DH=128), sharded
over heads across 8 NeuronCores (2 heads/core, tensor parallel). Each core
computes its heads' QKV projections, RoPE, causal softmax attention and the
partial output projection; the 8 partial [S, D] outputs are summed on host
(the all-reduce of the sharding hint).

v2 design (vs the 452us fp32r baseline):
- All matmul inputs in bf16 (1 cycle/row on the PE at ANY moving size, same
  as fp32r>=256, but: half the DMA bytes, 2-4x DVE element ops, and no
  min-256 moving-width constraint so diagonal blocks shrink to their true
  causal widths). PSUM accumulation stays fp32; measured rel err ~1e-2/2
  budget.
- No fp32->fp32r rounding copies: DMA lands bf16 directly.
- Softmax denominators: instead of ~88 ones-matmuls per head (each costing
  a full 512-row pass on the PE), probs tiles are folded on the DVE (bf16
  quad trees + one f32r running master per head/group, in-place partial
  width adds for the diagonal) and ONE ones-matmul per (head, group)
  computes the column sums. PE cost for Z drops ~30us.
- Phase interleaving: the per-block attention pipeline is exp-paced on the
  ACT engine (612ns/block vs 426ns of PE sim+PV work), so the projection
  matmuls of slice g+1 and the output projection of group g-1 are emitted
  as "fill" between the sweep blocks of group g. The PE never idles waiting
  for exp.
- PSUM: pacc(2: proj jt-pair accumulators) + pshared(3: sim/rope-P/vT/
  bc/outproj rotation) + ppvz(3: pv_h0, z_h0, pv_h1, z_h1 ring) = 8 banks,
  sized so consecutive groups/heads never serialize on banks.
- Startup: slice-0 x chunks and weight chunks DMA'd interleaved in exactly
  d-loop consumption order; first matmul at ~1.5us.
"""
import math
import numpy as np
import ml_dtypes
from collections import deque
from contextlib import ExitStack

import concourse.bass as bass
import concourse.tile as tile
from concourse import bacc, mybir
from concourse.bass_utils import run_bass_kernel_spmd

D, H, DH = 2048, 16, 128
NCORES = 8
HPC = H // NCORES  # 2 heads per core
ROPE_BASE = 10000.0
SCALE = 1.0 / math.sqrt(DH)
F32 = mybir.dt.float32
F32R = mybir.dt.float32r
BF16 = mybir.dt.bfloat16
Exp = mybir.ActivationFunctionType.Exp
BF = ml_dtypes.bfloat16

_BUILD_CACHE: dict = {}
TRACE = False
LAST_RESULT = None

# diagonal key-block p: query cols [128p, 512) are unmasked
DW = (512, 384, 256, 128)
DO = (0, 128, 256, 384)


def _build(S: int):
    assert S % 512 == 0
    NG = S // 512   # 8 query groups == s-slices
    ND = D // 128   # 16 contraction tiles
    NB = S // 128   # 32 key blocks

    nc = bacc.Bacc("TRN2", target_bir_lowering=False, debug=False)

    xT_d = nc.dram_tensor("xT", [D, S], BF16, kind="ExternalInput")
    w_d = nc.dram_tensor("wqkvT", [D, 768], BF16, kind="ExternalInput")
    wo_d = nc.dram_tensor("woT", [2 * DH, D], BF16, kind="ExternalInput")
    cs_d = nc.dram_tensor("cs", [128, 2 * S], BF16, kind="ExternalInput")
    cb_d = nc.dram_tensor("cb", [128, 2304], BF16, kind="ExternalInput")
    cr_d = nc.dram_tensor("cr", [128, 130], F32R, kind="ExternalInput")
    out_d = nc.dram_tensor("outp", [S, D], BF16, kind="ExternalOutput")

    with tile.TileContext(nc) as tc, ExitStack() as ctx:
        persist = ctx.enter_context(tc.tile_pool(name="persist", bufs=1))
        work = ctx.enter_context(tc.tile_pool(name="work", bufs=2))
        pacc = ctx.enter_context(tc.tile_pool(name="pacc", bufs=3, space="PSUM"))
        pshared = ctx.enter_context(tc.tile_pool(name="psh", bufs=3, space="PSUM"))
        ppv = ctx.enter_context(tc.tile_pool(name="ppv", bufs=2, space="PSUM"))

        # ---- persistent tiles ----
        w_r = persist.tile([128, ND * 768], BF16, tag="w", name="wr")
        wo_r = persist.tile([128, 2 * D], BF16, tag="wo", name="wor")
        kT = [persist.tile([128, S], BF16, tag=f"kT{h}", name=f"kT{h}")
              for h in range(2)]
        v_sb = persist.tile([128, NB * 256], BF16, tag="v", name="vsb")
        cb_t = persist.tile([128, 2304], BF16, tag="cb", name="cbt")
        cr_t = persist.tile([128, 130], F32R, tag="cr", name="crt")
        PT_b = cb_t[:, 0:128]
        ident_b = cb_t[:, 128:256]
        onescol_r = cr_t[:, 0:1]
        onesrow_r = cr_t[0:1, 1:129]
        masks = [cb_t[:, 256 + p * 512:256 + p * 512 + DW[p]] for p in range(4)]

        # ---- DMA issue helpers ----
        def issue_x(g, interleave_w=False):
            """DMA x chunks (+cos/sin) for slice g; optionally interleave the
            16 w chunks + cb in d-consumption order (startup)."""
            xs = []
            for dd in range(ND // 2):
                t = work.tile([128, 1024], BF16, tag="xs", bufs=18, name="xs")
                src = xT_d.ap()[dd * 256:(dd + 1) * 256, g * 512:(g + 1) * 512]
                nc.sync.dma_start(
                    t[:].rearrange("b (a c) -> b a c", a=2),
                    src.rearrange("(a b) c -> b a c", a=2),
                )
                xs.append(t)
                if interleave_w:
                    if dd == 0:
                        # singles so the very first matmul unblocks sooner
                        for d in (0, 1):
                            nc.sync.dma_start(
                                w_r[:, d * 768:(d + 1) * 768],
                                w_d.ap()[d * 128:(d + 1) * 128, :],
                            )
                    else:
                        # two w chunks in one strided DMA, consumption order
                        wdst = w_r[:, dd * 1536:(dd + 1) * 1536]
                        wsrc = w_d.ap()[dd * 256:(dd + 1) * 256, :]
                        nc.sync.dma_start(
                            wdst.rearrange("b (a c) -> b a c", a=2),
                            wsrc.rearrange("(a b) c -> b a c", a=2),
                        )
                    if dd == 1:
                        nc.sync.dma_start(cb_t[:], cb_d.ap())
                    if dd == 2:
                        nc.sync.dma_start(cr_t[:], cr_d.ap())
            cos = work.tile([128, 512], BF16, tag="cos", bufs=2, name="cos")
            sin = work.tile([128, 512], BF16, tag="sin", bufs=2, name="sin")
            nc.sync.dma_start(cos[:], cs_d.ap()[:, g * 512:(g + 1) * 512])
            nc.sync.dma_start(sin[:], cs_d.ap()[:, S + g * 512:S + (g + 1) * 512])
            return xs, cos, sin

        # ---- slice compute (projections + rope + vT), as a fill generator ----
        PAIRS = ((0, 1, "q"), (2, 3, "k"), (4, 5, "v"))

        def slice_steps(g, xs, cos, sin, qts, pairs=PAIRS, yield_every=1):
            pend = []  # deferred post-steps, run with >=2 d-steps of lag

            def rope_step(kind, hh, t_in):
                def run():
                    pps = pshared.tile([128, 512], F32, tag="sh", bufs=3,
                                       name="pps")
                    nc.tensor.matmul(pps[:], PT_b, t_in[:],
                                     start=True, stop=True,
                                     skip_group_check=True)
                    t1 = work.tile([128, 512], BF16, tag="t1", bufs=2,
                                   name="t1")
                    nc.gpsimd.tensor_mul(t1[:], t_in[:], cos[:])
                    t2 = work.tile([128, 512], BF16, tag="t2", bufs=2,
                                   name="t2")
                    nc.vector.tensor_mul(t2[:], pps[:], sin[:])
                    if kind == "q":
                        nc.gpsimd.tensor_add(qts[hh][:], t1[:], t2[:])
                    else:
                        nc.gpsimd.tensor_add(
                            kT[hh][:, g * 512:(g + 1) * 512], t1[:], t2[:])
                return run

            def vt_step(hh, vtmp):
                def run():
                    tp = pshared.tile([128, 512], BF16, tag="sh", bufs=3,
                                      name="vtp")
                    for t in range(4):
                        nc.tensor.matmul(
                            tp[:, t * 128:(t + 1) * 128],
                            vtmp[:, t * 128:(t + 1) * 128],
                            ident_b,
                            is_transpose=True, skip_group_check=True,
                        )
                    dst = v_sb[:].rearrange("p (b x) -> p b x", x=256)[
                        :, 4 * g:4 * g + 4, hh * 128:(hh + 1) * 128]
                    src = tp[:].rearrange("p (b x) -> p b x", x=128)
                    nc.vector.tensor_copy(dst, src)
                return run

            for (jta, jtb, kind) in pairs:
                acc = [pacc.tile([128, 512], F32, tag="acc", bufs=3, name="acc")
                       for _ in range(2)]
                for d in range(ND):
                    xr = xs[d // 2][:, (d % 2) * 512:(d % 2) * 512 + 512]
                    for i, jt in enumerate((jta, jtb)):
                        nc.tensor.matmul(
                            acc[i][:],
                            w_r[:, d * 768 + jt * 128:d * 768 + (jt + 1) * 128],
                            xr,
                            start=(d == 0), stop=(d == ND - 1),
                            skip_group_check=True,
                        )
                    if d % yield_every == yield_every - 1:
                        yield
                # drain the accs to SBUF (Pool + DVE in parallel); defer the
                # compute halves and give the drains two sweep-blocks of lead
                for hh in range(2):
                    t_in = work.tile([128, 512], BF16,
                                     tag="vtmp" if kind == "v" else "tin",
                                     bufs=4, name="tin")
                    nc.scalar.copy(t_in[:, 0:256], acc[hh][:, 0:256])
                    nc.vector.tensor_copy(t_in[:, 256:512],
                                          acc[hh][:, 256:512])
                    pend.append(vt_step(hh, t_in) if kind == "v"
                                else rope_step(kind, hh, t_in))
                # run the deferred steps at the pair boundary: they separate
                # this pair's acc drains from the next pair's bank reuse
                yield
                while pend:
                    pend.pop(0)()
                    yield
            while pend:
                pend.pop(0)()
                yield

        # ---- group tail (recip/bc/ot) + output projection, fill generators --
        def tail_steps(g, pvz):
            ots = []
            for hh in range(2):
                if hh == 0:
                    pv, rc = pvz[0]  # recip already done inline in sweep
                else:
                    pv, master = pvz[1]
                    zz = pshared.tile([1, 512], F32, tag="sh", bufs=3,
                                      name="z")
                    nc.tensor.matmul(zz[:], onescol_r, master[:],
                                     start=True, stop=True,
                                     skip_group_check=True)
                    rc = work.tile([1, 512], F32R, tag="rc", bufs=2, name="rc")
                    with nc.allow_low_precision(reason="fp32r rounding of 1/Z"):
                        nc.vector.reciprocal(rc[:], zz[:])
                bc = pshared.tile([128, 512], F32, tag="sh", bufs=3, name="bc")
                nc.tensor.matmul(bc[:], onesrow_r, rc[:], start=True, stop=True,
                                 skip_group_check=True)
                bcs = work.tile([128, 512], BF16, tag="bcs", bufs=2, name="bcs")
                nc.scalar.copy(bcs[:], bc[:])
                ot = work.tile([128, 512], BF16, tag=f"ot{hh}", bufs=2,
                               name=f"ot{hh}")
                nc.vector.tensor_mul(ot[:], pv[:], bcs[:])
                ots.append(ot)
                yield
            pvz.append(ots)  # hand ots to op_steps via shared list

        def op_steps(g, pvz):
            last = g == NG - 1
            while len(pvz) < 3:
                yield  # wait until tail_steps appended ots (same deque order)
            ots = pvz[2]
            for t in range(4):
                osb = work.tile([128, D], BF16, tag="osb", bufs=3, name="osb")
                for n in range(4):
                    op = pshared.tile([128, 512], F32, tag="sh", bufs=3,
                                      name="op")
                    for hh in range(2):
                        nc.tensor.matmul(
                            op[:],
                            ots[hh][:, t * 128:(t + 1) * 128],
                            wo_r[:, hh * D + n * 512:hh * D + (n + 1) * 512],
                            start=(hh == 0), stop=(hh == 1),
                            skip_group_check=True,
                        )
                    if last and n % 2 == 1:
                        nc.scalar.copy(osb[:, n * 512:(n + 1) * 512], op[:])
                    else:
                        nc.vector.tensor_copy(osb[:, n * 512:(n + 1) * 512],
                                              op[:])
                    yield
                nc.sync.dma_start(
                    out_d.ap()[g * 512 + t * 128:g * 512 + (t + 1) * 128, :],
                    osb[:],
                )

        # ---- fill machinery ----
        fill_q = deque()

        def pull_fill(n=1):
            for _ in range(n):
                while fill_q:
                    try:
                        next(fill_q[0])
                        break
                    except StopIteration:
                        fill_q.popleft()
                else:
                    return

        def drain_fill():
            while fill_q:
                pull_fill()

        # ---- attention sweep for group g (emits blocks, pulls fill) ----
        # Block order: both heads' non-diagonal blocks first (h0 then h1),
        # then the diagonal blocks of h0 and h1.  This pushes the kT/v_sb
        # dependency on slice g to the END of the sweep, so slice g's k/v
        # projection passes can serve as fill for sweep g itself (used for
        # the last group, whose sweep otherwise has no projection fill).
        def sweep(g, qts):
            nkb = 4 * g + 4
            pvz_out = []
            pv = [None, None]
            master = [None, None]
            quad = [[], []]
            nblk = [0, 0]

            def madd(hh, x_ap, o, w):
                if master[hh] is None:
                    assert o == 0 and w == 512
                    master[hh] = work.tile([128, 512], F32R, tag=f"m{hh}",
                                           bufs=2, name=f"m{hh}")
                    nc.vector.tensor_copy(master[hh][:], x_ap)
                elif o == 0 and w == 512:
                    nc.vector.tensor_add(master[hh][:], master[hh][:], x_ap)
                else:
                    nc.vector.tensor_add(master[hh][:, o:o + w],
                                         master[hh][:, o:o + w], x_ap)

            order = [(hh, j) for hh in range(2) for j in range(4 * g)] + \
                    [(hh, 4 * g + p) for hh in range(2) for p in range(4)]
            for (hh, j) in order:
                if pv[hh] is None:
                    pv[hh] = ppv.tile([128, 512], F32, tag="pv", bufs=2,
                                      name=f"pv{hh}")
                p = j - 4 * g
                diag = p >= 0
                o, w = (DO[p], DW[p]) if diag else (0, 512)
                sim = pshared.tile([128, 512], F32, tag="sh", bufs=3,
                                   name="sim")
                nc.tensor.matmul(
                    sim[:, 0:w],
                    kT[hh][:, j * 128:(j + 1) * 128],
                    qts[hh][:, o:512],
                    start=True, stop=True, skip_group_check=True,
                )
                pr = work.tile([128, 512], BF16, tag="pr", bufs=8,
                               name="pr")
                nc.scalar.activation(pr[:, 0:w], sim[:, 0:w], Exp,
                                     scale=SCALE)
                if diag:
                    nc.vector.tensor_mul(pr[:, 0:w], pr[:, 0:w], masks[p])
                nc.tensor.matmul(
                    pv[hh][:, o:512],
                    v_sb[:, j * 256 + hh * 128:j * 256 + hh * 128 + 128],
                    pr[:, 0:w],
                    start=(nblk[hh] == 0), stop=(nblk[hh] == nkb - 1),
                    skip_group_check=True,
                )
                nblk[hh] += 1
                # Z fold
                if not diag:
                    q = quad[hh]
                    q.append(pr)
                    if len(q) == 4:
                        s1 = work.tile([128, 512], BF16, tag="zf", bufs=4,
                                       name="zf")
                        nc.vector.tensor_add(s1[:], q[0][:], q[1][:])
                        s2 = work.tile([128, 512], BF16, tag="zf", bufs=4,
                                       name="zf")
                        nc.vector.tensor_add(s2[:], s1[:], q[2][:])
                        if master[hh] is None:
                            master[hh] = work.tile([128, 512], F32R,
                                                   tag=f"m{hh}", bufs=2,
                                                   name=f"m{hh}")
                            nc.vector.tensor_add(master[hh][:], s2[:],
                                                 q[3][:])
                        else:
                            s3 = work.tile([128, 512], BF16, tag="zf",
                                           bufs=4, name="zf")
                            nc.vector.tensor_add(s3[:], s2[:], q[3][:])
                            nc.vector.tensor_add(master[hh][:],
                                                 master[hh][:], s3[:])
                        quad[hh] = []
                else:
                    madd(hh, pr[:, 0:w], o, w)
                pull_fill(1)
                yield
                if nblk[0] == nkb and master[0] is not None and \
                        len(pvz_out) == 0:
                    # h0 finished all its blocks: emit its Z + recip now
                    pull_fill(3)
                    zz = pshared.tile([1, 512], F32, tag="sh", bufs=3,
                                      name="z")
                    nc.tensor.matmul(zz[:], onescol_r, master[0][:],
                                     start=True, stop=True,
                                     skip_group_check=True)
                    rc = work.tile([1, 512], F32R, tag="rc", bufs=2, name="rc")
                    with nc.allow_low_precision(reason="fp32r rounding of 1/Z"):
                        nc.vector.reciprocal(rc[:], zz[:])
                    pvz_out.append((pv[0], rc))
            assert not quad[0] and not quad[1]
            # h1's Z is emitted by tail_steps after h0's bc, hiding the
            # fold-chain tail behind h0's normalization
            pvz_out.append((pv[1], master[1]))
            return pvz_out

        # ================= main program =================
        xs0, cos0, sin0 = issue_x(0, interleave_w=True)
        nc.sync.dma_start(wo_r[:, 0:D], wo_d.ap()[0:128, :])
        nc.sync.dma_start(wo_r[:, D:2 * D], wo_d.ap()[128:256, :])

        qts_all = {}

        def new_qts():
            return [work.tile([128, 512], BF16, tag=f"qt{h}", bufs=2,
                              name=f"qt{h}") for h in range(2)]

        # slice 0 runs un-filled (nothing to overlap with yet)
        qts_all[0] = new_qts()
        for _ in slice_steps(0, xs0, cos0, sin0, qts_all[0]):
            pass

        for g in range(NG):
            if g + 1 < NG:
                xs_n, cos_n, sin_n = issue_x(g + 1)
                qts_all[g + 1] = new_qts()
                if g + 1 == NG - 1:
                    # last slice: only the q projection must precede sweep 7;
                    # its k/v passes become fill INSIDE sweep 7 (whose
                    # diagonal blocks, the only consumers, come last)
                    fill_q.append(slice_steps(g + 1, xs_n, cos_n, sin_n,
                                              qts_all[g + 1],
                                              pairs=PAIRS[:1]))
                    kv7 = slice_steps(g + 1, xs_n, cos_n, sin_n,
                                      qts_all[g + 1], pairs=PAIRS[1:],
                                      yield_every=2)
                else:
                    fill_q.append(
                        slice_steps(g + 1, xs_n, cos_n, sin_n,
                                    qts_all[g + 1]))
            # run the sweep (pulls fill: [tail g-1, op g-1, proj g+1])
            sw = sweep(g, qts_all[g])
            pvz = None
            try:
                while True:
                    next(sw)
            except StopIteration as e:
                pvz = e.value
            # everything queued must land before the next sweep's sims
            drain_fill()
            fill_q.append(tail_steps(g, pvz))
            fill_q.append(op_steps(g, pvz))
            if g == NG - 1:
                drain_fill()
            elif g + 1 == NG - 1:
                fill_q.append(kv7)

    nc.compile()
    return nc


def _host_tables(S: int):
    inv = 1.0 / (ROPE_BASE ** (np.arange(0, DH, 2, dtype=np.float64) / DH))
    t = np.arange(S, dtype=np.float64)
    fr = np.outer(t, inv)  # [S, 64]
    cos = np.repeat(np.cos(fr), 2, axis=1)
    sin = np.repeat(np.sin(fr), 2, axis=1)
    cs = np.concatenate([cos.T, sin.T], axis=1).astype(BF)  # [128, 2S]

    PT = np.zeros((DH, DH), np.float32)
    for m in range(DH // 2):
        PT[2 * m + 1, 2 * m] = -1.0
        PT[2 * m, 2 * m + 1] = 1.0
    cb = np.zeros((128, 2304), np.float32)
    cb[:, 0:128] = PT
    cb[:, 128:256] = np.eye(128, dtype=np.float32)
    # diagonal-block causal masks: tile col c (query 128p+c) vs key partition
    part = np.arange(128)[:, None]
    for p in range(4):
        w = (512, 384, 256, 128)[p]
        c = np.arange(w)[None, :]
        cb[:, 256 + p * 512:256 + p * 512 + w] = (c >= part).astype(np.float32)
    cb = cb.astype(BF)

    cr = np.zeros((128, 130), np.float32)
    cr[:, 0] = 1.0        # onescol
    cr[0, 1:129] = 1.0    # onesrow
    return cs, cb, cr


def kernel(x, mask, wq, wk, wv, wo):
    x = np.asarray(x, dtype=np.float32)
    wq = np.asarray(wq, dtype=np.float32)
    wk = np.asarray(wk, dtype=np.float32)
    wv = np.asarray(wv, dtype=np.float32)
    wo = np.asarray(wo, dtype=np.float32)
    S = x.shape[0]

    if S not in _BUILD_CACHE:
        _BUILD_CACHE[S] = _build(S)
    nc = _BUILD_CACHE[S]

    cs, cb, cr = _host_tables(S)
    xT = np.ascontiguousarray(x.T.astype(BF))

    in_maps = []
    for c in range(NCORES):
        hsl = slice(c * HPC * DH, (c + 1) * HPC * DH)
        wqT = wq[hsl].T.reshape(D, 2, DH)
        wkT = wk[hsl].T.reshape(D, 2, DH)
        wvT = wv[hsl].T.reshape(D, 2, DH)
        wqkvT = np.concatenate(
            [wqT[:, 0], wqT[:, 1], wkT[:, 0], wkT[:, 1], wvT[:, 0], wvT[:, 1]],
            axis=1,
        ).astype(BF)
        woT = np.ascontiguousarray(wo[:, hsl].T.astype(BF))
        in_maps.append(
            {
                "xT": xT,
                "wqkvT": np.ascontiguousarray(wqkvT),
                "woT": woT,
                "cs": cs,
                "cb": cb,
                "cr": cr,
            }
        )

    res = run_bass_kernel_spmd(
        nc, in_maps, core_ids=list(range(NCORES)), trace=TRACE
    )
    global LAST_RESULT
    LAST_RESULT = res
    out = np.zeros((S, D), np.float32)
    for r in res.results:
        out += np.asarray(r["outp"], dtype=np.float32)
    return out
